# revision 18
# baseline (speedup 1.0000x reference)
"""Trainium2 Bass kernel for nn_BiquadFilter.

Math: the reference builds, per batch, an 8192-tap FIR from 6 cascaded
biquads (frequency sampling: rfft of 3-tap coeff arrays -> cascade product
-> irfft), then linearly convolves each [C=2, L=524288] signal with it
(causal, truncated to L).

Device implementation (one batch per NeuronCore, 8 cores):
 1. Preamble (unchanged from the direct-conv version): tanh activations,
    frequency response wH on a [u=128, j=33] grid, 3-step irfft(8192)
    giving fir[n] laid out [n1=64, n2=128] (n = 128 n1 + n2) in PSUM.
 2. H = FFT_16384(fir)/16384 via a matmul Cooley-Tukey mini-FFT
    (128x128 split), kept on-chip in fp16 as [k2, k1] (f = k1 + 128 k2).
 3. Convolution by overlap-save FFT: the two channels are packed as one
    complex signal z = ch0 + i ch1 (the FIR is real, so
    irfft(fft(z) * H) yields both channels at once). 64 blocks of
    N=16384 (hop 8192), each a 2-stage matmul FFT with all contractions
    on the partition axis (no transposes):
      stage A  (PE): DFT-128 over n1, stationary = z block
      twiddle  (DVE fp16), after an ACT fp32->fp16 PSUM copy
      stage B  (PE): DFT-128 over n2, stationary = basis
      H-mult   (GpSimd fp16)
      stage B'' (PE): iDFT-128 over k2, stationary = spectrum
      inv twiddle (DVE fp16)
      stage A'' (PE): iDFT-128 over k1, stationary = basis
    Blocks are processed in pairs so the basis-stationary stages run
    512-wide and the pointwise ops amortize instruction overhead.
    Only 6 distinct fp16 basis/twiddle tables are needed (cos/sin
    symmetry makes the inverse tables aliases of the forward ones).
"""

import numpy as np

FIR_LEN = 8192
L = 524288
C = 2
B = 8
K = 6
NJ = 33                  # preamble f chunks (33*128 = 4224 >= 4097)
NQ = 64                  # fir rows (64*128 = 8192)
NBLK = 64                # overlap-save blocks (hop 8192, N = 16384)
NGRP = NBLK // 2         # blocks processed in pairs
ZW = 8192 + L            # padded stream length (532480)

_CACHE = {}


def _build_constants():
    f = np.arange(NJ * 128)
    w = np.zeros(NJ * 128, np.float64)
    w[0] = 1.0
    w[4096] = 1.0
    w[1:4096] = 2.0
    w /= FIR_LEN
    th = 2.0 * np.pi * f / FIR_LEN
    c1 = np.cos(th)
    s1 = -np.sin(th)
    c2 = np.cos(2 * th)
    s2 = -np.sin(2 * th)
    for a in (c1, s1, c2, s2):
        a[4097:] = 0.0
    w[4097:] = 0.0

    def t(a):
        return np.ascontiguousarray(a.reshape(NJ, 128).T.astype(np.float32))

    u = np.arange(128)
    p = np.arange(128)
    j = np.arange(NJ)
    q = np.arange(NQ)
    Are = np.cos(2 * np.pi * np.outer(u, p) / FIR_LEN).astype(np.float32)
    Aim = np.sin(2 * np.pi * np.outer(u, p) / FIR_LEN).astype(np.float32)
    Bre = np.cos(2 * np.pi * np.outer(j, p) / 64).astype(np.float32)
    Bim = np.sin(2 * np.pi * np.outer(j, p) / 64).astype(np.float32)
    Cre = np.cos(2 * np.pi * np.outer(u, q) / 64).astype(np.float32)
    Cim = np.sin(2 * np.pi * np.outer(u, q) / 64).astype(np.float32)
    CW = 5 * NJ + 128 * 4 + 64 * 2 + 128 * 3
    cpk = np.zeros((128, CW), np.float32)
    cpk[0, 0:128] = 1.0
    o = 128
    for a in (c1, s1, c2, s2, w):
        cpk[:, o:o + NJ] = t(a)
        o += NJ
    cpk[:, o:o + 128] = Are; o += 128
    cpk[:, o:o + 128] = Aim; o += 128
    cpk[:, o:o + 128] = np.eye(128, dtype=np.float32); o += 128
    cpk[:, o:o + NQ] = Cre; o += NQ
    cpk[:, o:o + NQ] = -Cim; o += NQ
    cpk[0:NJ, o:o + 128] = Bre; o += 128
    cpk[0:NJ, o:o + 128] = Bim; o += 128
    cpk[0:NJ, o:o + 128] = -Bim; o += 128

    # fp16 FFT tables.  DFT-128 bases c = cos, s = -sin, sn = +sin
    # (cos symmetry makes the inverse bases aliases).  Strips pack the
    # two movings used per stationary side by side so stages A/B'' need
    # one 256-wide matmul per stationary:
    #   strip1  = [c|s]   strip1b = [sn|c]   (stage A)
    #   strip2  = [c|sn]  strip2b = [s|c]    (stage B'')
    # T4* are the N=16384 twiddles tiled 4x for [Ar0|Ar1|Ai0|Ai1]-wide
    # fused pointwise ops.
    i128 = np.arange(128.0)
    th8 = 2 * np.pi * np.outer(i128, i128) / 128
    thT = 2 * np.pi * np.outer(i128, i128) / 16384
    c = np.cos(th8).astype(np.float16)
    s = (-np.sin(th8)).astype(np.float16)
    sn = np.sin(th8).astype(np.float16)
    Tr = np.cos(thT).astype(np.float16)
    Ti = (-np.sin(thT)).astype(np.float16)
    Tin = np.sin(thT).astype(np.float16)
    # T4i_pm = [-Ti|-Ti|Ti|Ti]: the second complex-mult product is taken
    # with the [Ar|Ai] halves swapped, so re/im combine with a single add.
    cpk16 = np.concatenate(
        [c, s, sn, c, c, sn, s, c] + [Tr] * 4
        + [-Ti, -Ti, Ti, Ti] + [-Tin, -Tin, Tin, Tin],
        axis=1).astype(np.float16)
    return {"cpk": cpk, "cpk16": np.ascontiguousarray(cpk16)}


def _build_program():
    import concourse.bass as bass
    import concourse.bacc as bacc
    import concourse.tile as tile
    from concourse import mybir

    F32 = mybir.dt.float32
    F16 = mybir.dt.float16
    ACT = mybir.ActivationFunctionType
    MUL = mybir.AluOpType.mult
    ADD = mybir.AluOpType.add
    SUB = mybir.AluOpType.subtract

    nc = bacc.Bacc("TRN2", target_bir_lowering=False, debug=False,
                   enable_asserts=False)

    coef_d = nc.dram_tensor("coef", [1, 30], F32, kind="ExternalInput")
    zt_d = nc.dram_tensor("zt", [NBLK, 128, 256], F16, kind="ExternalInput")
    CW = 5 * NJ + 128 * 4 + 64 * 2 + 128 * 3
    cpk_d = nc.dram_tensor("cpk", [128, CW], F32, kind="ExternalInput")
    cpk16_d = nc.dram_tensor("cpk16", [128, 20 * 128], F16,
                             kind="ExternalInput")

    yt_d = nc.dram_tensor("yt", [C, L], F32, kind="ExternalOutput")

    def bcast(ap_t, off, nk, nj_inner, k_is_inner):
        pstep = ap_t.ap[0][0]
        if k_is_inner:
            return bass.AP(tensor=ap_t.tensor, offset=ap_t.offset + off,
                           ap=[[pstep, 128], [1, nk], [0, nj_inner]])
        return bass.AP(tensor=ap_t.tensor, offset=ap_t.offset + off,
                       ap=[[pstep, 128], [0, nk], [1, nj_inner]])

    with tile.TileContext(nc) as tc:
        with (
            tc.tile_pool(name="const", bufs=1) as cpool,
            tc.tile_pool(name="work", bufs=2) as work,
            tc.tile_pool(name="out", bufs=3) as outp,
            tc.tile_pool(name="psA", bufs=2, space="PSUM") as psA,
            tc.tile_pool(name="psX", bufs=2, space="PSUM") as psX,
            tc.tile_pool(name="psP", bufs=2, space="PSUM") as psP,
            tc.tile_pool(name="psY", bufs=1, space="PSUM") as psY,
            tc.tile_pool(name="pps", bufs=1, space="PSUM") as pps,
        ):
            # ---- input DMAs: coef first (heads the ring), constants on
            # separate rings so nothing queues behind the big tables ----
            sc = cpool.tile([1, 30], F32, tag="sc")
            nc.sync.dma_start(sc[:], coef_d.ap())
            cpk = cpool.tile([128, CW], F32, tag="cpk")
            nc.sync.dma_start(cpk[:, 0:128], cpk_d.ap()[:, 0:128])
            t16 = cpool.tile([128, 20 * 128], F16, tag="cpk16")
            nc.scalar.dma_start(t16[:], cpk16_d.ap())
            nc.gpsimd.dma_start(cpk[:, 128:CW], cpk_d.ap()[:, 128:CW])
            ones = cpk[0:1, 0:128]
            o = 128
            tabs = {}
            for n in ("c1", "s1", "c2", "s2", "wt"):
                tabs[n] = cpk[:, o:o + NJ]
                o += NJ
            Are = cpk[:, o:o + 128]; o += 128
            Aim = cpk[:, o:o + 128]; o += 128
            ident = cpk[:, o:o + 128]; o += 128
            Cre = cpk[:, o:o + NQ]; o += NQ
            Cimn = cpk[:, o:o + NQ]; o += NQ
            Bre = cpk[0:NJ, o:o + 128]; o += 128
            Bim = cpk[0:NJ, o:o + 128]; o += 128
            Bimn = cpk[0:NJ, o:o + 128]; o += 128

            c16 = t16[:, 0:128]
            s16 = t16[:, 128:256]
            strip1 = t16[:, 0:256]          # [c|s]
            sn16 = t16[:, 256:384]
            strip1b = t16[:, 256:512]       # [sn|c]
            strip2 = t16[:, 512:768]        # [c|sn]
            strip2b = t16[:, 768:1024]      # [s|c]
            T4r = t16[:, 1024:1536]
            T4ipm = t16[:, 1536:2048]
            T4inpm = t16[:, 2048:2560]
            Tr16 = T4r[:, 0:128]
            Ti16 = T4ipm[:, 256:384]      # the +Ti block

            H4r = cpool.tile([128, 512], F16, tag="H4r")
            H4ipm = cpool.tile([128, 512], F16, tag="H4ipm")

            def PP():
                return pps.tile([128, 512], F32, tag="pp", name="pp")

            # ======== preamble chunks (emitted interleaved with conv) ======
            pre = {}

            def c0():
                th = cpool.tile([1, 12], F32, tag="th")
                nc.scalar.activation(th[:], sc[:, 18:30], ACT.Tanh)
                ab = cpool.tile([1, 6], F32, tag="ab")
                nc.scalar.activation(ab[:], th[:, 0:6], ACT.Abs)
                scal = cpool.tile([1, 30], F32, tag="scal")
                nc.vector.tensor_copy(scal[:, 0:18], sc[:, 0:18])
                nc.vector.tensor_scalar_mul(scal[:, 18:24], th[:, 0:6], 2.0)
                tm = cpool.tile([1, 6], F32, tag="tm")
                nc.vector.tensor_mul(tm[:], ab[:], th[:, 6:12])
                x3 = cpool.tile([1, 6], F32, tag="x3")
                nc.vector.tensor_add(x3[:], th[:, 6:12], ab[:])
                nc.vector.tensor_sub(scal[:, 24:30], x3[:], tm[:])
                bc_ps = PP()
                nc.tensor.matmul(bc_ps[:, 0:30], ones, scal[:],
                                 start=True, stop=True,
                                 skip_group_check=True)
                bc = cpool.tile([128, 30], F32, tag="bc_sb")
                nc.vector.tensor_copy(bc[:], bc_ps[:, 0:30])
                pre["bc"] = bc

            def allk(basis_a, basis_b, o1, o2, extra, otag):
                bc = pre["bc"]
                t1 = work.tile([128, K * NJ], F32, tag=otag + "t1",
                               name=otag + "t1")
                nc.vector.tensor_tensor(
                    t1[:].rearrange("u (k j) -> u k j", k=K),
                    bcast(basis_a, 0, K, NJ, False),
                    bcast(bc[:], o1, K, NJ, True), MUL)
                t2 = work.tile([128, K * NJ], F32, tag=otag + "t2",
                               name=otag + "t2")
                nc.vector.tensor_tensor(
                    t2[:].rearrange("u (k j) -> u k j", k=K),
                    bcast(basis_b, 0, K, NJ, False),
                    bcast(bc[:], o2, K, NJ, True), MUL)
                ot = work.tile([128, K * NJ], F32, tag=otag, name=otag)
                nc.vector.tensor_add(ot[:], t1[:], t2[:])
                if extra == "b0":
                    nc.vector.tensor_tensor(
                        ot[:].rearrange("u (k j) -> u k j", k=K),
                        ot[:].rearrange("u (k j) -> u k j", k=K),
                        bcast(bc[:], 0, K, NJ, True), ADD)
                elif extra == "one":
                    nc.vector.tensor_scalar_add(ot[:], ot[:], 1.0)
                return ot

            def c1():
                c1t, s1t = tabs["c1"], tabs["s1"]
                c2t, s2t = tabs["c2"], tabs["s2"]
                pre["bfre"] = allk(c1t, c2t, 6, 12, "b0", "bfre")
                pre["bfim"] = allk(s1t, s2t, 6, 12, None, "bfim")
                pre["afre"] = allk(c1t, c2t, 18, 24, "one", "afre")
                pre["afim"] = allk(s1t, s2t, 18, 24, None, "afim")

            def cmul_slices(re_t, im_t, lo0, lo1, n, otag):
                w_ = n * NJ
                a_re = re_t[:, lo0 * NJ:(lo0 + n) * NJ]
                a_im = im_t[:, lo0 * NJ:(lo0 + n) * NJ]
                b_re = re_t[:, lo1 * NJ:(lo1 + n) * NJ]
                b_im = im_t[:, lo1 * NJ:(lo1 + n) * NJ]
                t1 = work.tile([128, w_], F32, tag="ct1", name="ct1")
                nc.vector.tensor_mul(t1[:], a_re, b_re)
                t2 = work.tile([128, w_], F32, tag="ct2", name="ct2")
                nc.vector.tensor_mul(t2[:], a_im, b_im)
                orr = work.tile([128, w_], F32, tag=otag + "re",
                                name=otag + "re")
                nc.vector.tensor_sub(orr[:], t1[:], t2[:])
                nc.vector.tensor_mul(t1[:], a_re, b_im)
                nc.vector.tensor_mul(t2[:], a_im, b_re)
                oi = work.tile([128, w_], F32, tag=otag + "im",
                               name=otag + "im")
                nc.vector.tensor_add(oi[:], t1[:], t2[:])
                return orr, oi

            def cascade(re_t, im_t, otag):
                p3re, p3im = cmul_slices(re_t, im_t, 0, 3, 3, otag + "3")
                q1re, q1im = cmul_slices(p3re, p3im, 0, 1, 1, otag + "q")
                t1 = work.tile([128, NJ], F32, tag="ct1", name="ct1b")
                nc.vector.tensor_mul(t1[:], q1re[:], p3re[:, 2 * NJ:3 * NJ])
                t2 = work.tile([128, NJ], F32, tag="ct2", name="ct2b")
                nc.vector.tensor_mul(t2[:], q1im[:], p3im[:, 2 * NJ:3 * NJ])
                orr = work.tile([128, NJ], F32, tag=otag + "re",
                                name=otag + "fre")
                nc.vector.tensor_sub(orr[:], t1[:], t2[:])
                nc.vector.tensor_mul(t1[:], q1re[:], p3im[:, 2 * NJ:3 * NJ])
                nc.vector.tensor_mul(t2[:], q1im[:], p3re[:, 2 * NJ:3 * NJ])
                oi = work.tile([128, NJ], F32, tag=otag + "im",
                               name=otag + "fim")
                nc.vector.tensor_add(oi[:], t1[:], t2[:])
                return orr, oi

            def c2():
                pre["num"] = cascade(pre["bfre"], pre["bfim"], "num")

            def c3():
                pre["den"] = cascade(pre["afre"], pre["afim"], "den")

            def c4():
                numre, numim = pre["num"]
                denre, denim = pre["den"]
                d1 = work.tile([128, NJ], F32, tag="d1")
                nc.vector.tensor_mul(d1[:], denre[:], denre[:])
                d2 = work.tile([128, NJ], F32, tag="d2")
                nc.vector.tensor_mul(d2[:], denim[:], denim[:])
                dd = work.tile([128, NJ], F32, tag="dd")
                nc.vector.tensor_add(dd[:], d1[:], d2[:])
                rcp = work.tile([128, NJ], F32, tag="rcp")
                nc.vector.reciprocal(rcp[:], dd[:])
                wrcp = work.tile([128, NJ], F32, tag="wrcp")
                nc.vector.tensor_mul(wrcp[:], rcp[:], tabs["wt"])

                def hpart(t1in, t2in, sub, tagp):
                    t1 = work.tile([128, NJ], F32, tag="h1", name="h1")
                    nc.vector.tensor_mul(t1[:], t1in[0][:], t1in[1][:])
                    t2 = work.tile([128, NJ], F32, tag="h2", name="h2")
                    nc.vector.tensor_mul(t2[:], t2in[0][:], t2in[1][:])
                    hs = work.tile([128, NJ], F32, tag=tagp + "s",
                                   name=tagp + "s")
                    if sub:
                        nc.vector.tensor_sub(hs[:], t1[:], t2[:])
                    else:
                        nc.vector.tensor_add(hs[:], t1[:], t2[:])
                    ot = work.tile([128, NJ], F32, tag=tagp, name=tagp)
                    nc.vector.tensor_mul(ot[:], hs[:], wrcp[:])
                    return ot

                pre["wHre"] = hpart((numre, denre), (numim, denim),
                                    False, "wHre")
                pre["wHim"] = hpart((numim, denre), (numre, denim),
                                    True, "wHim")

            def c5():
                whT_ps = PP()
                nc.tensor.transpose(whT_ps[0:NJ, 0:128], pre["wHre"][:],
                                    ident)
                nc.tensor.transpose(whT_ps[0:NJ, 128:256], pre["wHim"][:],
                                    ident)
                whreT = work.tile([NJ, 128], F32, tag="whreTs")
                nc.vector.tensor_copy(whreT[:], whT_ps[0:NJ, 0:128])
                whimT = work.tile([NJ, 128], F32, tag="whimTs")
                nc.vector.tensor_copy(whimT[:], whT_ps[0:NJ, 128:256])
                tt_ps = PP()
                tre_ps = tt_ps[:, 0:128]
                tim_ps = tt_ps[:, 128:256]
                nc.tensor.matmul(tre_ps, whreT[:], Bre,
                                 start=True, stop=False,
                                 skip_group_check=True)
                nc.tensor.matmul(tre_ps, whimT[:], Bimn,
                                 start=False, stop=True,
                                 skip_group_check=True)
                nc.tensor.matmul(tim_ps, whreT[:], Bim,
                                 start=True, stop=False,
                                 skip_group_check=True)
                nc.tensor.matmul(tim_ps, whimT[:], Bre,
                                 start=False, stop=True,
                                 skip_group_check=True)
                u1 = work.tile([128, 128], F32, tag="u1")
                nc.vector.tensor_mul(u1[:], Are, tre_ps)
                u2 = work.tile([128, 128], F32, tag="u2")
                nc.vector.tensor_mul(u2[:], Aim, tim_ps)
                ure = work.tile([128, 128], F32, tag="ure")
                nc.vector.tensor_sub(ure[:], u1[:], u2[:])
                nc.vector.tensor_mul(u1[:], Are, tim_ps)
                nc.vector.tensor_mul(u2[:], Aim, tre_ps)
                uim = work.tile([128, 128], F32, tag="uim")
                nc.vector.tensor_add(uim[:], u1[:], u2[:])
                pre["u"] = (ure, uim)

            def c6():
                ure, uim = pre["u"]
                fir_ps = PP()
                nc.tensor.matmul(fir_ps[0:NQ, 0:128], Cre, ure[:],
                                 start=True, stop=False,
                                 skip_group_check=True)
                nc.tensor.matmul(fir_ps[0:NQ, 0:128], Cimn, uim[:],
                                 start=False, stop=True,
                                 skip_group_check=True)
                fir16 = cpool.tile([NQ, 128], F16, tag="fir16")
                nc.scalar.copy(fir16[:], fir_ps[0:NQ, 0:128])
                ha_ps = PP()
                nc.tensor.matmul(ha_ps[:, 0:128], fir16[:], c16[0:NQ, :],
                                 start=True, stop=True,
                                 skip_group_check=True)
                nc.tensor.matmul(ha_ps[:, 128:256], fir16[:], s16[0:NQ, :],
                                 start=True, stop=True,
                                 skip_group_check=True)
                har = work.tile([128, 128], F16, tag="har16")
                nc.scalar.copy(har[:], ha_ps[:, 0:128])
                hai = work.tile([128, 128], F16, tag="hai16")
                nc.scalar.copy(hai[:], ha_ps[:, 128:256])
                pre["ha"] = (har, hai)

            def c7():
                har, hai = pre["ha"]
                ht1 = work.tile([128, 128], F16, tag="ht1")
                nc.vector.tensor_mul(ht1[:], har[:], Tr16)
                ht2 = work.tile([128, 128], F16, tag="ht2")
                nc.vector.tensor_mul(ht2[:], hai[:], Ti16)
                hzr = work.tile([128, 128], F16, tag="hzr")
                nc.vector.tensor_sub(hzr[:], ht1[:], ht2[:])
                nc.vector.tensor_mul(ht1[:], har[:], Ti16)
                nc.vector.tensor_mul(ht2[:], hai[:], Tr16)
                hzi = work.tile([128, 128], F16, tag="hzi")
                nc.vector.tensor_add(hzi[:], ht1[:], ht2[:])
                hx_ps = PP()
                nc.tensor.matmul(hx_ps[:, 128:256], s16, hzr[:],
                                 start=True, stop=False,
                                 skip_group_check=True)
                nc.tensor.matmul(hx_ps[:, 128:256], c16, hzi[:],
                                 start=False, stop=True,
                                 skip_group_check=True)
                nc.tensor.matmul(hx_ps[:, 0:128], c16, hzr[:],
                                 start=True, stop=False,
                                 skip_group_check=True)
                nc.tensor.matmul(hx_ps[:, 0:128], sn16, hzi[:],
                                 start=False, stop=True,
                                 skip_group_check=True)
                r4 = lambda ap: ap.rearrange("p (b j) -> p b j", b=4)
                r2a = lambda ap: ap.rearrange("p (b j) -> p b j", b=2)
                nc.vector.tensor_scalar_mul(
                    r4(H4r[:]), bcast(hx_ps[:, 0:128], 0, 4, 128, False),
                    1.0 / 16384.0)
                nc.vector.tensor_scalar_mul(
                    r2a(H4ipm[:, 0:256]),
                    bcast(hx_ps[:, 128:256], 0, 2, 128, False),
                    -1.0 / 16384.0)
                nc.vector.tensor_scalar_mul(
                    r2a(H4ipm[:, 256:512]),
                    bcast(hx_ps[:, 128:256], 0, 2, 128, False),
                    1.0 / 16384.0)

            chunks = [c0, c1, c2, c3, c4, c5, c6, c7]

            # ================= overlap-save FFT convolution =================
            st = {}

            def swap_ap(t):
                a = t[:]
                return bass.AP(tensor=a.tensor, offset=a.offset + 256,
                               ap=[[a.ap[0][0], 128], [-256, 2],
                                   [1, 256]])

            def s_in(g):
                zb0 = work.tile([128, 256], F16, tag="zb0", name="zb0",
                                bufs=3)
                nc.gpsimd.dma_start(zb0[:], zt_d.ap()[2 * g])
                zb1 = work.tile([128, 256], F16, tag="zb1", name="zb1",
                                bufs=3)
                nc.gpsimd.dma_start(zb1[:], zt_d.ap()[2 * g + 1])
                st[g] = {"zb": (zb0, zb1)}

            def s_A(g):
                d = st[g]
                pA = psA.tile([128, 512], F32, tag="pA")
                for blk in (0, 1):
                    zb = d["zb"][blk]
                    dst = bass.AP(tensor=pA.tensor,
                                  offset=pA.offset + blk * 128,
                                  ap=[[pA.ap[0][0], 128],
                                      [256, 2], [1, 128]])
                    nc.tensor.matmul(dst, zb[:, 0:128], strip1,
                                     start=True, stop=False,
                                     skip_group_check=True)
                    nc.tensor.matmul(dst, zb[:, 128:256], strip1b,
                                     start=False, stop=True,
                                     skip_group_check=True)
                d["pA"] = pA

            def s_AC(g):
                d = st[g]
                a16 = work.tile([128, 512], F16, tag="a16", bufs=3)
                nc.scalar.copy(a16[:], d.pop("pA")[:])
                d["a16"] = a16

            def s_MM(g):
                d = st[g]
                a16 = d.pop("a16")
                m1 = work.tile([128, 512], F16, tag="m1", name="m1",
                               bufs=4)
                nc.vector.tensor_mul(m1[:], a16[:], T4r)
                m2 = work.tile([128, 512], F16, tag="m2", name="m2",
                               bufs=4)
                nc.vector.tensor_tensor(m2[:], swap_ap(a16), T4ipm,
                                        MUL)
                d["Z"] = (m1, m2)

            def s_B(g):
                d = st[g]
                m1, m2 = d.pop("Z")
                pX = psX.tile([128, 512], F32, tag="pX")
                nc.tensor.matmul(pX[:], c16, m1[:],
                                 start=True, stop=False,
                                 skip_group_check=True)
                nc.tensor.matmul(pX[:], c16, m2[:],
                                 start=False, stop=False,
                                 skip_group_check=True)
                nc.tensor.matmul(pX[:, 0:256], sn16, m1[:, 256:512],
                                 start=False, stop=False,
                                 skip_group_check=True)
                nc.tensor.matmul(pX[:, 0:256], sn16, m2[:, 256:512],
                                 start=False, stop=True,
                                 skip_group_check=True)
                nc.tensor.matmul(pX[:, 256:512], s16, m1[:, 0:256],
                                 start=False, stop=False,
                                 skip_group_check=True)
                nc.tensor.matmul(pX[:, 256:512], s16, m2[:, 0:256],
                                 start=False, stop=True,
                                 skip_group_check=True)
                d["pX"] = pX

            def s_XC(g):
                d = st[g]
                x16 = work.tile([128, 512], F16, tag="x16", bufs=7)
                nc.scalar.copy(x16[:], d.pop("pX")[:])
                d["x16"] = x16

            def s_HM(g):
                d = st[g]
                x16 = d.pop("x16")
                g1 = work.tile([128, 512], F16, tag="g1", name="g1",
                               bufs=5)
                nc.vector.tensor_mul(g1[:], x16[:], H4r[:])
                g2 = work.tile([128, 512], F16, tag="g2", name="g2",
                               bufs=5)
                nc.vector.tensor_tensor(g2[:], swap_ap(x16), H4ipm[:],
                                        MUL)
                d["g"] = (g1, g2)

            def s_P(g):
                d = st[g]
                g1, g2 = d.pop("g")
                pP = psP.tile([128, 512], F32, tag="pP")
                for blk in (0, 1):
                    hr = slice(blk * 128, blk * 128 + 128)
                    hi = slice(256 + blk * 128, 256 + blk * 128 + 128)
                    dst = bass.AP(tensor=pP.tensor,
                                  offset=pP.offset + blk * 128,
                                  ap=[[pP.ap[0][0], 128],
                                      [256, 2], [1, 128]])
                    nc.tensor.matmul(dst, g1[:, hr], strip2,
                                     start=True, stop=False,
                                     skip_group_check=True)
                    nc.tensor.matmul(dst, g2[:, hr], strip2,
                                     start=False, stop=False,
                                     skip_group_check=True)
                    nc.tensor.matmul(dst, g1[:, hi], strip2b,
                                     start=False, stop=False,
                                     skip_group_check=True)
                    nc.tensor.matmul(dst, g2[:, hi], strip2b,
                                     start=False, stop=True,
                                     skip_group_check=True)
                d["pP"] = pP

            def s_PC(g):
                d = st[g]
                p16 = work.tile([128, 512], F16, tag="p16", bufs=4)
                nc.scalar.copy(p16[:], d.pop("pP")[:])
                d["p16"] = p16

            def s_IM(g):
                d = st[g]
                p16 = d.pop("p16")
                im1 = work.tile([128, 512], F16, tag="im1", name="im1",
                                bufs=5)
                nc.vector.tensor_mul(im1[:], p16[:], T4r)
                im2 = work.tile([128, 512], F16, tag="im2", name="im2",
                                bufs=5)
                nc.vector.tensor_tensor(im2[:], swap_ap(p16), T4inpm,
                                        MUL)
                d["Y"] = (im1, im2)

            def s_Y(g):
                d = st.pop(g)
                im1, im2 = d["Y"]
                pY = psY.tile([128, 512], F32, tag="pY")
                nc.tensor.matmul(pY[:], c16, im1[:],
                                 start=True, stop=False,
                                 skip_group_check=True)
                nc.tensor.matmul(pY[:], c16, im2[:],
                                 start=False, stop=False,
                                 skip_group_check=True)
                nc.tensor.matmul(pY[:, 0:256], s16, im1[:, 256:512],
                                 start=False, stop=False,
                                 skip_group_check=True)
                nc.tensor.matmul(pY[:, 0:256], s16, im2[:, 256:512],
                                 start=False, stop=True,
                                 skip_group_check=True)
                nc.tensor.matmul(pY[:, 256:512], sn16, im1[:, 0:256],
                                 start=False, stop=False,
                                 skip_group_check=True)
                nc.tensor.matmul(pY[:, 256:512], sn16, im2[:, 0:256],
                                 start=False, stop=True,
                                 skip_group_check=True)
                ysb = outp.tile([64, 512], F32, tag="ysb")
                nc.scalar.copy(ysb[:], pY[64:128, :])
                for ch in (0, 1):
                    dst = bass.AP(tensor=yt_d,
                                  offset=ch * L + 8192 * 2 * g,
                                  ap=[[128, 64], [8192, 2], [1, 128]])
                    nc.sync.dma_start(dst, ysb[:, ch * 256:ch * 256 + 256])

            # (fn, steady-state offset, first body allowed to emit).
            # H-dependent stages start late (preamble), then catch up by
            # emitting up to 2 iterations per body until back to the tight
            # offsets, which shortens the drain tail.
            sched = [[s_in, 0, 0, 0], [s_A, 1, 0, 0], [s_AC, 2, 0, 0],
                     [s_MM, 3, 0, 0], [s_B, 5, 0, 0], [s_XC, 6, 0, 0],
                     [s_HM, 7, 11, 0], [s_P, 9, 13, 0], [s_PC, 10, 14, 0],
                     [s_IM, 11, 15, 0], [s_Y, 13, 17, 0]]
            chunk_at = {0: c0, 1: c1, 2: c2, 3: c3, 4: c4,
                        7: c5, 8: c6, 9: c7}
            t = 0
            while any(s[3] < NGRP for s in sched):
                if t in chunk_at:
                    chunk_at[t]()
                for s in sched:
                    fn, off, t0, nxt = s
                    if t < t0:
                        continue
                    g_hi = min(t - off, NGRP - 1)
                    g_emit = min(g_hi, nxt + 1)
                    while s[3] <= g_emit:
                        fn(s[3])
                        s[3] += 1
                t += 1

    nc.compile()
    return nc


def _get_program():
    if "nc" not in _CACHE:
        _CACHE["nc"] = _build_program()
        _CACHE["consts"] = _build_constants()
    return _CACHE["nc"], _CACHE["consts"]


def _prep_core_inputs(consts, x_b, Bs_b, A1_b, A2_b):
    # z = ch0 + i ch1, padded with 8192 leading zeros, pre-blocked into
    # overlapping [64, 128, 256] fp16 windows (cols 0:128 re, 128:256 im)
    z = np.zeros((2, ZW), np.float16)
    z[0, 8192:] = x_b[0]
    z[1, 8192:] = x_b[1]
    zt = np.empty((NBLK, 128, 256), np.float16)
    for b in range(NBLK):
        blk = z[:, 8192 * b:8192 * b + 16384].reshape(2, 128, 128)
        zt[b, :, 0:128] = blk[0]
        zt[b, :, 128:256] = blk[1]
    coef = np.concatenate(
        [Bs_b[:, 0], Bs_b[:, 1], Bs_b[:, 2], A1_b, A2_b]
    ).astype(np.float32).reshape(1, 30)
    m = {"zt": zt, "coef": coef}
    m.update(consts)
    return m


def kernel(input_signal, Bs, A1_pre, A2_pre):
    from concourse import bass_utils

    nc, consts = _get_program()
    input_signal = np.asarray(input_signal, dtype=np.float32)
    Bs = np.asarray(Bs, dtype=np.float32)
    A1_pre = np.asarray(A1_pre, dtype=np.float32)
    A2_pre = np.asarray(A2_pre, dtype=np.float32)

    in_maps = [
        _prep_core_inputs(consts, input_signal[b], Bs[b], A1_pre[b], A2_pre[b])
        for b in range(B)
    ]
    res = bass_utils.run_bass_kernel_spmd(nc, in_maps, core_ids=list(range(B)))
    out = np.empty((B, C, L), np.float32)
    for b in range(B):
        out[b] = res.results[b]["yt"]                  # [C, L]
    return out
